# revision 24
# baseline (speedup 1.0000x reference)
"""Mixtral MoE layer (top-2 of 8 experts) as a Trainium2 Bass/Tile kernel.

Strategy (expert-parallel + selective fp8):
  - 8 NeuronCores, one expert per core. Host replays the router in fp32
    numpy to decide token->core sharding AND per-(token, expert) routing
    weights; the device does NO routing.
  - Selective precision: per expert, tokens are sorted by routing weight.
    The top (L_e - nf_e) tokens run all-bf16 (as the proven baseline);
    the lowest-weight nf_e tokens run ALL THREE GEMMs in fp8 e4m3 with
    MatmulPerfMode.DoubleRow (two K-planes of 128 contracted per
    instruction => 2x PE throughput, probe-verified on HW at the same
    per-instruction issue rate as bf16). fp8 error per assignment scales
    with its routing weight; the host scans the bf16 capacity NB at
    runtime so the predicted rel err stays under ERR_TARGET using a
    sim-calibrated quadrature model (err^2 = base^2 + rho * K_ALL3,
    rho = w^2-share of fp8 assignments; model verified exact vs CPU sim).
    Balance-aware selection: nf_e = (L_e - NB)+ takes fp8 tokens from the
    most-loaded experts first, so device cost = NB + maxNF/2 beats the
    pure-bf16 cost of max L_e (~1080 -> ~948 token-equivalents).
  - All bf16 GEMMs keep fp32 PSUM accumulation; fp8 path quantizes x,
    w1/w3/w2 and the gated activation g = silu(h1)*h3 to e4m3 (silu via
    scalar engine -> bf16 staging, DVE mult writes fp8 directly;
    probe-verified bit-exact with ml_dtypes.float8_e4m3 rounding).
  - Device phases: warmup (PE clock-gate release) -> bf16-A -> bf16-B ->
    fp8-A -> fp8-B. Critical input + output DMAs on the Sync queue in
    consumption order (first w1 chunk split in h-halves so the first G1
    chain's operands land as warmup ends); fp8 weight chunks go on the
    gpsimd (SWDGE) queue so their pool-recycle stalls park an idle queue.
  - NOTE on clocks: each process is assigned a device that runs the PE at
    either ~2.37 GHz or ~1.97 GHz (observed ~50/50 per-process lottery,
    exact 1.2x ratio; both ladders release once from base 1.2/1.0 GHz
    after ~8 busy warmup matmuls). The program is identical in both; all
    timings here scale with that bin.
"""

from contextlib import ExitStack

import ml_dtypes
import numpy as np

import concourse.bacc as bacc
import concourse.tile as tile
from concourse import mybir
from concourse.bass_utils import run_bass_kernel_spmd

P = 128
AF = mybir.ActivationFunctionType
OP = mybir.AluOpType
F32 = mybir.dt.float32
BF16 = mybir.dt.bfloat16
FP8 = mybir.dt.float8e4
DRMODE = mybir.MatmulPerfMode.DoubleRow
E4 = ml_dtypes.float8_e4m3

H = 1024
F = 3584
HT = H // P          # 8 h-tiles (contraction, bf16)
HP = HT // 2         # 4 h-pair DR tiles (contraction, fp8)
FT = F // P          # 28 f-tiles
FP2 = FT // 2        # 14 f-pair DR tiles
FCH = 4              # f-tiles per weight chunk
NCH = FT // FCH      # 7 chunks
CMAXBUILD = 1408     # max bf16 capacity for the fallback dense program

# fp8 error model (CPU-sim calibrated on the problem's randn data):
#   err^2 = ERR_BASE^2 + rho * K_ALL3,  rho = sel w^2 / total w^2
K_ALL3 = 5.15e-3
ERR_BASE = 4.5e-3
ERR_TARGET = 1.84e-2


def _slices(C, cap=512):
    """Balanced 8-aligned column slices of width <= cap covering C."""
    nsl = -(-C // cap)
    k8 = C // 8
    out, off = [], 0
    for i in range(nsl):
        w = (k8 // nsl + (1 if i < k8 % nsl else 0)) * 8
        out.append((off, w))
        off += w
    assert off == C
    return out


def build_moe_nc(C=1088, WU=16, NF=0):
    """Single-core SPMD program: bf16 width C, optional fp8-DR width NF."""
    assert C % 8 == 0 and C <= CMAXBUILD
    assert NF % 8 == 0 and NF <= 512
    nw_slices = _slices(C)

    nc = bacc.Bacc("TRN2", target_bir_lowering=False, debug=False)
    xt_d = nc.dram_tensor("xt", [P, HT, C], BF16, kind="ExternalInput").ap()
    w1_d = nc.dram_tensor("w1c", [P, NCH, HT, FCH * P], BF16,
                          kind="ExternalInput").ap()
    w3_d = nc.dram_tensor("w3c", [P, NCH, HT, FCH * P], BF16,
                          kind="ExternalInput").ap()
    w2_d = nc.dram_tensor("w2c", [P, FT, H], BF16, kind="ExternalInput").ap()
    out_d = nc.dram_tensor("out", [H, C], F32, kind="ExternalOutput").ap()
    if NF:
        xq_d = nc.dram_tensor("xq", [P, HP, 2, NF], FP8,
                              kind="ExternalInput").ap()
        w1q_d = nc.dram_tensor("w1q", [P, NCH, HP, FCH, 2, P], FP8,
                               kind="ExternalInput").ap()
        w3q_d = nc.dram_tensor("w3q", [P, NCH, HP, FCH, 2, P], FP8,
                               kind="ExternalInput").ap()
        w2q_d = nc.dram_tensor("w2q", [P, FP2, HT, 2, P], FP8,
                               kind="ExternalInput").ap()
        outf_d = nc.dram_tensor("outf", [H, NF], F32,
                                kind="ExternalOutput").ap()

    with tile.TileContext(nc) as tc, ExitStack() as ctx:
        x_pool = ctx.enter_context(tc.tile_pool(name="x", bufs=1))
        s1_pool = ctx.enter_context(tc.tile_pool(name="s1", bufs=FCH))
        w13_pool = ctx.enter_context(tc.tile_pool(name="w13", bufs=3))
        w2_pool = ctx.enter_context(tc.tile_pool(name="w2", bufs=1))
        g_pool = ctx.enter_context(tc.tile_pool(name="g", bufs=FT))
        ob_pool = ctx.enter_context(tc.tile_pool(name="ob", bufs=2))
        wu_pool = ctx.enter_context(tc.tile_pool(name="wu", bufs=2))
        ps12 = ctx.enter_context(tc.tile_pool(name="ps12", bufs=6, space="PSUM"))
        ps3 = ctx.enter_context(tc.tile_pool(name="ps3", bufs=2, space="PSUM"))
        if NF:
            xq_pool = ctx.enter_context(tc.tile_pool(name="xq", bufs=1))
            w13q_pool = ctx.enter_context(tc.tile_pool(name="w13q", bufs=3))
            w2q_pool = ctx.enter_context(tc.tile_pool(name="w2q", bufs=1))
            gf_pool = ctx.enter_context(tc.tile_pool(name="gf", bufs=FP2))
            obf_pool = ctx.enter_context(tc.tile_pool(name="obf", bufs=2))

        # ---- PE warm-up: dummy matmuls while the first DMAs land, so the
        # HAM clock gate releases before the first real GEMM.
        wu_w = wu_pool.tile([P, P], BF16, tag="wu")
        wu_x = wu_pool.tile([P, 512], BF16, tag="wu2")
        nc.vector.memset(wu_w[:], 0.0)
        nc.vector.memset(wu_x[:], 0.0)
        wu_ps = ps3.tile([P, 512], F32, tag="ps3", name="wu_ps")
        for i in range(WU):
            nc.tensor.matmul(wu_ps[:], wu_w[:], wu_x[:],
                             start=(i == 0), stop=(i == WU - 1))

        # ---- input DMAs, in consumption order. Everything critical is
        # queued before any DMA that can stall on pool recycling.
        w1c, w3c = [], []

        def load_w(lst, d, c):
            t_ = w13_pool.tile([P, HT, FCH * P], BF16, tag="w13",
                               name=f"w_{len(lst)}_{c}")
            nc.sync.dma_start(out=t_[:], in_=d[:, c, :, :])
            lst.append(t_)

        # first w1 chunk in h-halves interleaved with the x stream so the
        # first G1 chain's operands land right as the warmup ends
        w1c0 = w13_pool.tile([P, HT, FCH * P], BF16, tag="w13", name="w_0_0")
        nc.sync.dma_start(out=w1c0[:, 0:4, :], in_=w1_d[:, 0, 0:4, :])
        xt = x_pool.tile([P, HT, C], BF16, tag="x")
        nc.sync.dma_start(out=xt[:, 0:2, :], in_=xt_d[:, 0:2, :])
        nc.sync.dma_start(out=xt[:, 2:4, :], in_=xt_d[:, 2:4, :])
        nc.sync.dma_start(out=w1c0[:, 4:8, :], in_=w1_d[:, 0, 4:8, :])
        nc.sync.dma_start(out=xt[:, 4:6, :], in_=xt_d[:, 4:6, :])
        nc.sync.dma_start(out=xt[:, 6:8, :], in_=xt_d[:, 6:8, :])
        w1c.append(w1c0)
        load_w(w3c, w3_d, 0)
        for c in range(1, 4):
            load_w(w1c, w1_d, c)
            load_w(w3c, w3_d, c)
        w2t = w2_pool.tile([P, FT, H], BF16, tag="w2")
        nc.sync.dma_start(out=w2t[:], in_=w2_d[:, :, :])
        if NF:
            xq = xq_pool.tile([P, HP, 2, NF], FP8, tag="xq")
            nc.sync.dma_start(out=xq[:], in_=xq_d[:])
            w2qt = w2q_pool.tile([P, FP2, HT, 2, P], FP8, tag="w2q")
            nc.sync.dma_start(out=w2qt[:], in_=w2q_d[:])
        for c in range(4, NCH):
            load_w(w1c, w1_d, c)
            load_w(w3c, w3_d, c)

        # ---- bf16 phase A: h1T/h3T per f-tile + silu*mul -> resident g
        g_tiles = []
        for c in range(NCH):
            s1_tiles = []
            for fq in range(FCH):
                p1 = [ps12.tile([P, w], F32, tag="ps12", name=f"p1_{c}_{fq}_{s}")
                      for s, (o, w) in enumerate(nw_slices)]
                for h in range(HT):
                    lw = w1c[c][:, h, fq * P:(fq + 1) * P]
                    for s, (o, w) in enumerate(nw_slices):
                        nc.tensor.matmul(
                            p1[s][:], lw, xt[:, h, o:o + w],
                            start=(h == 0), stop=(h == HT - 1))
                s1 = s1_pool.tile([P, C], BF16, tag="s1")
                for s, (o, w) in enumerate(nw_slices):
                    nc.scalar.activation(s1[:, o:o + w], p1[s][:], AF.Silu)
                s1_tiles.append(s1)
            for fq in range(FCH):
                p3 = [ps12.tile([P, w], F32, tag="ps12", name=f"p3_{c}_{fq}_{s}")
                      for s, (o, w) in enumerate(nw_slices)]
                for h in range(HT):
                    lw = w3c[c][:, h, fq * P:(fq + 1) * P]
                    for s, (o, w) in enumerate(nw_slices):
                        nc.tensor.matmul(
                            p3[s][:], lw, xt[:, h, o:o + w],
                            start=(h == 0), stop=(h == HT - 1))
                gt = g_pool.tile([P, C], BF16, tag="g")
                for s, (o, w) in enumerate(nw_slices):
                    nc.vector.tensor_tensor(
                        gt[:, o:o + w], s1_tiles[fq][:, o:o + w], p3[s][:],
                        OP.mult)
                g_tiles.append(gt)

        if NF:
            # fp8 weight chunks go on the gpsimd (SWDGE) DMA queue: pool
            # recycling stalls park that idle queue instead of the Sync
            # queue, which must keep streaming the bf16-B/fp8-B out DMAs.
            w1qc, w3qc = [], []

            def load_wq(lst, d, c):
                t_ = w13q_pool.tile([P, HP, FCH, 2, P], FP8, tag="w13q",
                                    name=f"wq_{len(lst)}_{c}")
                nc.gpsimd.dma_start(out=t_[:], in_=d[:, c, :, :, :, :])
                lst.append(t_)

            for c in range(NCH):
                load_wq(w1qc, w1q_d, c)
                load_wq(w3qc, w3q_d, c)

        # ---- bf16 phase B: outT[H-part, tok] over all 28 f-tiles in one
        # PSUM chain per (h-tile, token-slice).
        tail_slices = []
        toff = 0
        while C - toff > 512:
            tail_slices.append((toff, 512))
            toff += 512
        tail_slices.append((toff, C - toff))
        for hh in range(HT):
            ob = ob_pool.tile([P, C], F32, tag="ob")
            hs = tail_slices if hh == HT - 1 else nw_slices
            for s, (o, w) in enumerate(hs):
                po = ps3.tile([P, w], F32, tag="ps3", name=f"po_{hh}_{s}")
                for fi in range(FT):
                    nc.tensor.matmul(
                        po[:], w2t[:, fi, hh * P:(hh + 1) * P],
                        g_tiles[fi][:, o:o + w],
                        start=(fi == 0), stop=(fi == FT - 1))
                nc.scalar.copy(ob[:, o:o + w], po[:])
                if hh == HT - 1:
                    nc.sync.dma_start(
                        out=out_d[hh * P:(hh + 1) * P, o:o + w],
                        in_=ob[:, o:o + w])
            if hh < HT - 1:
                nc.sync.dma_start(out=out_d[hh * P:(hh + 1) * P, :], in_=ob[:])

        if NF:
            # ---- fp8 phase A: DR h1/h3 over 4 h-pair planes per f-tile.
            gf_tiles = [gf_pool.tile([P, 2, NF], FP8, tag="gf",
                                     name=f"gf_{i}") for i in range(FP2)]
            # 4+4 grouping (G1 for all fq, then G3): tolerant of late w3q
            # chunk DMAs. The 8 live PSUM tiles are spread across all 8
            # banks (4 p1f on ps12, p3f on 2 ps3 + 2 ps12) so no chain
            # start ever waits on a silu/mult eviction.
            for c in range(NCH):
                s1f_tiles = []
                for fq in range(FCH):
                    p1f = ps12.tile([P, NF], F32, tag="ps12",
                                    name=f"p1f_{c}_{fq}")
                    for hp in range(HP):
                        nc.tensor.matmul(
                            p1f[:], w1qc[c][:, hp, fq, :, :], xq[:, hp, :, :],
                            start=(hp == 0), stop=(hp == HP - 1),
                            perf_mode=DRMODE)
                    s1f = s1_pool.tile([P, NF], BF16, tag="s1",
                                       name=f"s1f_{c}_{fq}")
                    nc.scalar.activation(s1f[:], p1f[:], AF.Silu)
                    s1f_tiles.append(s1f)
                for fq in range(FCH):
                    p3f = ps12.tile([P, NF], F32, tag="ps12",
                                    name=f"p3f_{c}_{fq}")
                    for hp in range(HP):
                        nc.tensor.matmul(
                            p3f[:], w3qc[c][:, hp, fq, :, :], xq[:, hp, :, :],
                            start=(hp == 0), stop=(hp == HP - 1),
                            perf_mode=DRMODE)
                    fi = c * FCH + fq
                    nc.vector.tensor_tensor(
                        gf_tiles[fi // 2][:, fi % 2, :], s1f_tiles[fq][:],
                        p3f[:], OP.mult)

            # ---- fp8 phase B: DR chain over 14 f-pairs per h-tile.
            for hh in range(HT):
                # alternate pools (ps12 idle in this phase): 4 chains in
                # flight so no chain waits on a prior chain's eviction
                pool_h = ps12 if hh % 2 else ps3
                pof = pool_h.tile([P, NF], F32,
                                  tag="ps12" if hh % 2 else "ps3",
                                  name=f"pof_{hh}")
                for fp_i in range(FP2):
                    nc.tensor.matmul(
                        pof[:], w2qt[:, fp_i, hh, :, :], gf_tiles[fp_i][:],
                        start=(fp_i == 0), stop=(fp_i == FP2 - 1),
                        perf_mode=DRMODE)
                obf = obf_pool.tile([P, NF], F32, tag="obf", name=f"obf_{hh}")
                nc.scalar.copy(obf[:], pof[:])
                nc.sync.dma_start(out=outf_d[hh * P:(hh + 1) * P, :],
                                  in_=obf[:])

    nc.compile()
    return nc


_NC_CACHE = {}


def _get_nc(key, **kw):
    if key not in _NC_CACHE:
        _NC_CACHE[key] = build_moe_nc(**kw)
    return _NC_CACHE[key]


def _host_route(x2, gate_w):
    """Host replay of the router: token lists + routing weights per expert."""
    logits = x2.astype(np.float32) @ gate_w.astype(np.float32).T
    order = np.argsort(-logits, axis=1, kind="stable")[:, :2]
    m = logits.max(axis=1, keepdims=True)
    ex = np.exp(logits - m)
    p = ex / ex.sum(axis=1, keepdims=True)
    T = logits.shape[0]
    p12 = p[np.arange(T)[:, None], order]           # [T, 2]
    p12 = p12 / p12.sum(axis=1, keepdims=True)
    E = gate_w.shape[0]
    idx, wts = [], []
    for e in range(E):
        sel = order == e                             # [T, 2]
        tok = np.nonzero(sel.any(axis=1))[0]
        w = np.where(sel[tok, 0], p12[tok, 0], p12[tok, 1]).astype(np.float32)
        idx.append(tok)
        wts.append(w)
    return idx, wts


def _host_top2_idx(x2, gate_w):
    """Back-compat helper for test.py: token index list per expert."""
    return _host_route(x2, gate_w)[0]


def _select_fp8(idx, wts):
    """Scan bf16 capacity NB: lowest NB whose predicted err <= ERR_TARGET.

    Per expert the lowest-weight (L_e - NB)+ assignments go fp8.
    Returns (NB, NF) padded to 8; NF == 0 means pure bf16.
    """
    loads = np.array([len(i) for i in idx])
    w2tot = sum(float((w.astype(np.float64) ** 2).sum()) for w in wts)
    pref = [np.concatenate([[0.0], np.cumsum(np.sort(w.astype(np.float64) ** 2))])
            for w in wts]
    budget = max(0.0, ERR_TARGET ** 2 - ERR_BASE ** 2)
    lmax = int(loads.max())
    best = (-(-lmax // 8) * 8, 0)
    for nb in range(-(-lmax // 8) * 8 - 8, 0, -8):
        nf = np.maximum(0, loads - nb)
        selw2 = sum(pref[e][min(int(nf[e]), len(wts[e]))]
                    for e in range(len(loads)))
        if w2tot <= 0 or (selw2 / w2tot) * K_ALL3 > budget:
            break
        nfp = -(-int(nf.max()) // 8) * 8
        if nfp > 512:
            break
        best = (nb, nfp)
    return best


def _prep_weights(w1, w2, w3, want_fp8):
    """Per-expert device weight layouts (bf16 + optional fp8 DR packs)."""
    E = w1.shape[0]
    maps = []
    for e in range(E):
        w1t = np.asarray(w1[e], np.float32).T        # [H, F]
        w3t = np.asarray(w3[e], np.float32).T        # [H, F]
        w2t = np.asarray(w2[e], np.float32).T        # [F, H]
        m = {
            "w1c": np.ascontiguousarray(
                w1t.astype(ml_dtypes.bfloat16)
                .reshape(HT, P, NCH, FCH * P).transpose(1, 2, 0, 3)),
            "w3c": np.ascontiguousarray(
                w3t.astype(ml_dtypes.bfloat16)
                .reshape(HT, P, NCH, FCH * P).transpose(1, 2, 0, 3)),
            "w2c": np.ascontiguousarray(
                w2t.astype(ml_dtypes.bfloat16)
                .reshape(FT, P, H).transpose(1, 0, 2)),
        }
        if want_fp8:
            # stationary DR packs: plane i of pair contracts h/f-tile 2k+i
            m["w1q"] = np.ascontiguousarray(
                w1t.astype(E4).reshape(HP, 2, P, NCH, FCH, P)
                .transpose(2, 3, 0, 4, 1, 5))
            m["w3q"] = np.ascontiguousarray(
                w3t.astype(E4).reshape(HP, 2, P, NCH, FCH, P)
                .transpose(2, 3, 0, 4, 1, 5))
            m["w2q"] = np.ascontiguousarray(
                w2t.astype(E4).reshape(FP2, 2, P, HT, P)
                .transpose(2, 0, 3, 1, 4))
        maps.append(m)
    return maps


def kernel(hidden_states, gate_w, w1, w2, w3, _trace=False, _trace_kwargs=None):
    B, S, Hh = hidden_states.shape
    assert Hh == H
    E = gate_w.shape[0]
    T = B * S
    x2 = np.asarray(hidden_states, dtype=np.float32).reshape(T, H)
    idx, wts = _host_route(x2, gate_w)
    xbf = x2.astype(ml_dtypes.bfloat16)

    NB, NF = _select_fp8(idx, wts)
    cmax = max(len(i) for i in idx)
    out = np.zeros((T, H), dtype=np.float32)

    if NF == 0 or NB > CMAXBUILD:
        # fallback: pure-bf16 path (pathological imbalance -> multi-launch)
        wmaps = _prep_weights(w1, w2, w3, want_fp8=False)
        nlaunch = -(-cmax // CMAXBUILD)
        per = -(-cmax // nlaunch)
        C = max(512, -(-per // 8) * 8)
        nc = _get_nc(("bf16", C), C=C)
        for li in range(nlaunch):
            in_maps = []
            for e in range(E):
                tok = idx[e][li * C:(li + 1) * C]
                xg = np.zeros((C, H), dtype=ml_dtypes.bfloat16)
                xg[:len(tok)] = xbf[tok]
                m = dict(wmaps[e])
                m["xt"] = np.ascontiguousarray(
                    xg.T.reshape(HT, P, C).transpose(1, 0, 2))
                in_maps.append(m)
            res = run_bass_kernel_spmd(
                nc, in_maps, list(range(E)), trace=_trace,
                **(_trace_kwargs or {}))
            kernel.last_results = res
            for e, r in enumerate(res.results):
                tok = idx[e][li * C:(li + 1) * C]
                w = wts[e][li * C:(li + 1) * C]
                out[tok] += r["out"][:, :len(tok)].T * w[:, None]
        return out.reshape(B, S, H).astype(hidden_states.dtype)

    wmaps = _prep_weights(w1, w2, w3, want_fp8=True)
    nc = _get_nc(("fp8", NB, NF), C=NB, NF=NF)
    in_maps = []
    hi_sel, lo_sel = [], []
    for e in range(E):
        order_w = np.argsort(wts[e], kind="stable")   # ascending weight
        nf_e = max(0, len(idx[e]) - NB)
        lo = order_w[:nf_e]
        hi = order_w[nf_e:]
        hi_sel.append(hi)
        lo_sel.append(lo)
        xg = np.zeros((NB, H), dtype=ml_dtypes.bfloat16)
        xg[:len(hi)] = xbf[idx[e][hi]]
        xql = np.zeros((NF, H), dtype=E4)
        xql[:len(lo)] = x2[idx[e][lo]].astype(E4)
        m = dict(wmaps[e])
        m["xt"] = np.ascontiguousarray(
            xg.T.reshape(HT, P, NB).transpose(1, 0, 2))
        # xq[p, hp, i, t] = x_lo.T[(2hp+i)*128+p, t]
        m["xq"] = np.ascontiguousarray(
            xql.T.reshape(HP, 2, P, NF).transpose(2, 0, 1, 3))
        in_maps.append(m)
    res = run_bass_kernel_spmd(
        nc, in_maps, list(range(E)), trace=_trace, **(_trace_kwargs or {}))
    kernel.last_results = res
    for e, r in enumerate(res.results):
        hi, lo = hi_sel[e], lo_sel[e]
        if len(hi):
            out[idx[e][hi]] += r["out"][:, :len(hi)].T * wts[e][hi][:, None]
        if len(lo):
            out[idx[e][lo]] += r["outf"][:, :len(lo)].T * wts[e][lo][:, None]
    return out.reshape(B, S, H).astype(hidden_states.dtype)


# revision 27
# speedup vs baseline: 1.0187x; 1.0187x over previous
"""Mixtral MoE layer (top-2 of 8 experts) as a Trainium2 Bass/Tile kernel.

Strategy (expert-parallel + selective fp8):
  - 8 NeuronCores, one expert per core. Host replays the router in fp32
    numpy to decide token->core sharding AND per-(token, expert) routing
    weights; the device does NO routing.
  - Selective precision: per expert, tokens are sorted by routing weight.
    The top (L_e - nf_e) tokens run all-bf16 (as the proven baseline);
    the lowest-weight nf_e tokens run ALL THREE GEMMs in fp8 e4m3 with
    MatmulPerfMode.DoubleRow (two K-planes of 128 contracted per
    instruction => 2x PE throughput, probe-verified on HW at the same
    per-instruction issue rate as bf16). fp8 error per assignment scales
    with its routing weight; the host scans the bf16 capacity NB at
    runtime so the predicted rel err stays under ERR_TARGET using a
    sim-calibrated quadrature model (err^2 = base^2 + rho * K_ALL3,
    rho = w^2-share of fp8 assignments; model verified exact vs CPU sim).
    Balance-aware selection: nf_e = (L_e - NB)+ takes fp8 tokens from the
    most-loaded experts first, so device cost = NB + maxNF/2 beats the
    pure-bf16 cost of max L_e (~1080 -> ~948 token-equivalents).
  - All bf16 GEMMs keep fp32 PSUM accumulation; fp8 path quantizes x,
    w1/w3/w2 and the gated activation g = silu(h1)*h3 to e4m3 (silu via
    scalar engine -> bf16 staging, DVE mult writes fp8 directly;
    probe-verified bit-exact with ml_dtypes.float8_e4m3 rounding).
  - Device phases: warmup (PE clock-gate release) -> bf16-A -> bf16-B ->
    fp8-A -> fp8-B. Critical input + output DMAs on the Sync queue in
    consumption order (first w1 chunk split in h-halves so the first G1
    chain's operands land as warmup ends); fp8 weight chunks go on the
    gpsimd (SWDGE) queue so their pool-recycle stalls park an idle queue.
  - NOTE on clocks: each process is assigned a device that runs the PE at
    either ~2.37 GHz or ~1.97 GHz (observed ~50/50 per-process lottery,
    exact 1.2x ratio; both ladders release once from base 1.2/1.0 GHz
    after ~8 busy warmup matmuls). The program is identical in both; all
    timings here scale with that bin.
"""

from contextlib import ExitStack

import ml_dtypes
import numpy as np

import concourse.bacc as bacc
import concourse.tile as tile
from concourse import mybir
from concourse.bass_utils import run_bass_kernel_spmd

P = 128
AF = mybir.ActivationFunctionType
OP = mybir.AluOpType
F32 = mybir.dt.float32
BF16 = mybir.dt.bfloat16
FP8 = mybir.dt.float8e4
DRMODE = mybir.MatmulPerfMode.DoubleRow
E4 = ml_dtypes.float8_e4m3

H = 1024
F = 3584
HT = H // P          # 8 h-tiles (contraction, bf16)
HP = HT // 2         # 4 h-pair DR tiles (contraction, fp8)
FT = F // P          # 28 f-tiles
FP2 = FT // 2        # 14 f-pair DR tiles
FCH = 4              # f-tiles per weight chunk
NCH = FT // FCH      # 7 chunks
CMAXBUILD = 1408     # max bf16 capacity for the fallback dense program

# fp8 error model (CPU-sim calibrated on the problem's randn data):
#   err^2 = ERR_BASE^2 + rho * K_ALL3,  rho = sel w^2 / total w^2
K_ALL3 = 5.15e-3
ERR_BASE = 4.5e-3
ERR_TARGET = 1.80e-2


def _slices(C, cap=512):
    """Balanced 8-aligned column slices of width <= cap covering C."""
    nsl = -(-C // cap)
    k8 = C // 8
    out, off = [], 0
    for i in range(nsl):
        w = (k8 // nsl + (1 if i < k8 % nsl else 0)) * 8
        out.append((off, w))
        off += w
    assert off == C
    return out


def build_moe_nc(C=1088, WU=16, NF=0):
    """Single-core SPMD program: bf16 width C, optional fp8-DR width NF."""
    assert C % 8 == 0 and C <= CMAXBUILD
    assert NF % 8 == 0 and NF <= 512
    nw_slices = _slices(C)

    nc = bacc.Bacc("TRN2", target_bir_lowering=False, debug=False)
    xt_d = nc.dram_tensor("xt", [P, HT, C], BF16, kind="ExternalInput").ap()
    w1_d = nc.dram_tensor("w1c", [P, NCH, HT, FCH * P], BF16,
                          kind="ExternalInput").ap()
    w3_d = nc.dram_tensor("w3c", [P, NCH, HT, FCH * P], BF16,
                          kind="ExternalInput").ap()
    w2_d = nc.dram_tensor("w2c", [P, FT, H], BF16, kind="ExternalInput").ap()
    out_d = nc.dram_tensor("out", [H, C], F32, kind="ExternalOutput").ap()
    if NF:
        xq_d = nc.dram_tensor("xq", [P, HP, 2, NF], FP8,
                              kind="ExternalInput").ap()
        w1q_d = nc.dram_tensor("w1q", [P, NCH, HP, FCH, 2, P], FP8,
                               kind="ExternalInput").ap()
        w3q_d = nc.dram_tensor("w3q", [P, NCH, HP, FCH, 2, P], FP8,
                               kind="ExternalInput").ap()
        w2q_d = nc.dram_tensor("w2q", [P, FP2, HT, 2, P], FP8,
                               kind="ExternalInput").ap()
        outf_d = nc.dram_tensor("outf", [H, NF], F32,
                                kind="ExternalOutput").ap()

    with tile.TileContext(nc) as tc, ExitStack() as ctx:
        x_pool = ctx.enter_context(tc.tile_pool(name="x", bufs=1))
        s1_pool = ctx.enter_context(tc.tile_pool(name="s1", bufs=FCH))
        w13_pool = ctx.enter_context(tc.tile_pool(name="w13", bufs=3))
        w2_pool = ctx.enter_context(tc.tile_pool(name="w2", bufs=1))
        g_pool = ctx.enter_context(tc.tile_pool(name="g", bufs=FT))
        ob_pool = ctx.enter_context(tc.tile_pool(name="ob", bufs=2))
        wu_pool = ctx.enter_context(tc.tile_pool(name="wu", bufs=2))
        ps12 = ctx.enter_context(tc.tile_pool(name="ps12", bufs=6, space="PSUM"))
        ps3 = ctx.enter_context(tc.tile_pool(name="ps3", bufs=2, space="PSUM"))
        if NF:
            xq_pool = ctx.enter_context(tc.tile_pool(name="xq", bufs=1))
            w13q_pool = ctx.enter_context(tc.tile_pool(name="w13q", bufs=4))
            w2q_pool = ctx.enter_context(tc.tile_pool(name="w2q", bufs=1))
            gf_pool = ctx.enter_context(tc.tile_pool(name="gf", bufs=FP2))

        # ---- PE warm-up: dummy matmuls while the first DMAs land, so the
        # HAM clock gate releases before the first real GEMM.
        wu_w = wu_pool.tile([P, P], BF16, tag="wu")
        wu_x = wu_pool.tile([P, 512], BF16, tag="wu2")
        nc.vector.memset(wu_w[:], 0.0)
        nc.vector.memset(wu_x[:], 0.0)
        wu_ps = ps3.tile([P, 512], F32, tag="ps3", name="wu_ps")
        for i in range(WU):
            nc.tensor.matmul(wu_ps[:], wu_w[:], wu_x[:],
                             start=(i == 0), stop=(i == WU - 1))

        # ---- input DMAs, in consumption order. Everything critical is
        # queued before any DMA that can stall on pool recycling.
        w1c, w3c = [], []

        def load_w(lst, d, c):
            t_ = w13_pool.tile([P, HT, FCH * P], BF16, tag="w13",
                               name=f"w_{len(lst)}_{c}")
            nc.sync.dma_start(out=t_[:], in_=d[:, c, :, :])
            lst.append(t_)

        # first w1 chunk in h-halves interleaved with the x stream so the
        # first G1 chain's operands land right as the warmup ends
        w1c0 = w13_pool.tile([P, HT, FCH * P], BF16, tag="w13", name="w_0_0")
        nc.sync.dma_start(out=w1c0[:, 0:4, :], in_=w1_d[:, 0, 0:4, :])
        xt = x_pool.tile([P, HT, C], BF16, tag="x")
        nc.sync.dma_start(out=xt[:, 0:2, :], in_=xt_d[:, 0:2, :])
        nc.sync.dma_start(out=xt[:, 2:4, :], in_=xt_d[:, 2:4, :])
        nc.sync.dma_start(out=w1c0[:, 4:8, :], in_=w1_d[:, 0, 4:8, :])
        nc.sync.dma_start(out=xt[:, 4:6, :], in_=xt_d[:, 4:6, :])
        nc.sync.dma_start(out=xt[:, 6:8, :], in_=xt_d[:, 6:8, :])
        w1c.append(w1c0)
        load_w(w3c, w3_d, 0)
        for c in range(1, 4):
            load_w(w1c, w1_d, c)
            load_w(w3c, w3_d, c)
        w2t = w2_pool.tile([P, FT, H], BF16, tag="w2")
        nc.sync.dma_start(out=w2t[:], in_=w2_d[:, :, :])
        if NF:
            xq = xq_pool.tile([P, HP, 2, NF], FP8, tag="xq")
            nc.sync.dma_start(out=xq[:], in_=xq_d[:])
            w2qt = w2q_pool.tile([P, FP2, HT, 2, P], FP8, tag="w2q")
            nc.sync.dma_start(out=w2qt[:], in_=w2q_d[:])
        for c in range(4, NCH):
            load_w(w1c, w1_d, c)
            load_w(w3c, w3_d, c)

        # ---- bf16 phase A: h1T/h3T per f-tile + silu*mul -> resident g
        g_tiles = []
        for c in range(NCH):
            s1_tiles = []
            for fq in range(FCH):
                p1 = [ps12.tile([P, w], F32, tag="ps12", name=f"p1_{c}_{fq}_{s}")
                      for s, (o, w) in enumerate(nw_slices)]
                for h in range(HT):
                    lw = w1c[c][:, h, fq * P:(fq + 1) * P]
                    for s, (o, w) in enumerate(nw_slices):
                        nc.tensor.matmul(
                            p1[s][:], lw, xt[:, h, o:o + w],
                            start=(h == 0), stop=(h == HT - 1))
                s1 = s1_pool.tile([P, C], BF16, tag="s1")
                for s, (o, w) in enumerate(nw_slices):
                    nc.scalar.activation(s1[:, o:o + w], p1[s][:], AF.Silu)
                s1_tiles.append(s1)
            for fq in range(FCH):
                p3 = [ps12.tile([P, w], F32, tag="ps12", name=f"p3_{c}_{fq}_{s}")
                      for s, (o, w) in enumerate(nw_slices)]
                for h in range(HT):
                    lw = w3c[c][:, h, fq * P:(fq + 1) * P]
                    for s, (o, w) in enumerate(nw_slices):
                        nc.tensor.matmul(
                            p3[s][:], lw, xt[:, h, o:o + w],
                            start=(h == 0), stop=(h == HT - 1))
                gt = g_pool.tile([P, C], BF16, tag="g")
                for s, (o, w) in enumerate(nw_slices):
                    nc.vector.tensor_tensor(
                        gt[:, o:o + w], s1_tiles[fq][:, o:o + w], p3[s][:],
                        OP.mult)
                g_tiles.append(gt)

        if NF:
            # fp8 weight chunks go on the gpsimd (SWDGE) DMA queue: pool
            # recycling stalls park that idle queue instead of the Sync
            # queue, which must keep streaming the bf16-B/fp8-B out DMAs.
            w1qc, w3qc = [], []

            def load_wq(lst, d, c):
                t_ = w13q_pool.tile([P, HP, FCH, 2, P], FP8, tag="w13q",
                                    name=f"wq_{len(lst)}_{c}")
                nc.gpsimd.dma_start(out=t_[:], in_=d[:, c, :, :, :, :])
                lst.append(t_)

            for c in range(NCH):
                load_wq(w1qc, w1q_d, c)
                load_wq(w3qc, w3q_d, c)

        # ---- bf16 phase B: outT[H-part, tok] over all 28 f-tiles in one
        # PSUM chain per (h-tile, token-slice).
        tail_slices = []
        toff = 0
        while C - toff > 512:
            tail_slices.append((toff, 512))
            toff += 512
        tail_slices.append((toff, C - toff))
        for hh in range(HT):
            ob = ob_pool.tile([P, C], F32, tag="ob")
            hs = tail_slices if hh == HT - 1 else nw_slices
            for s, (o, w) in enumerate(hs):
                po = ps3.tile([P, w], F32, tag="ps3", name=f"po_{hh}_{s}")
                for fi in range(FT):
                    nc.tensor.matmul(
                        po[:], w2t[:, fi, hh * P:(hh + 1) * P],
                        g_tiles[fi][:, o:o + w],
                        start=(fi == 0), stop=(fi == FT - 1))
                nc.scalar.copy(ob[:, o:o + w], po[:])
                if hh == HT - 1:
                    nc.sync.dma_start(
                        out=out_d[hh * P:(hh + 1) * P, o:o + w],
                        in_=ob[:, o:o + w])
            if hh < HT - 1:
                nc.sync.dma_start(out=out_d[hh * P:(hh + 1) * P, :], in_=ob[:])

        if NF:
            # ---- fp8 phase A: DR h1/h3 over 4 h-pair planes per f-tile.
            gf_tiles = [gf_pool.tile([P, 2, NF], FP8, tag="gf",
                                     name=f"gf_{i}") for i in range(FP2)]
            # 4+4 grouping (G1 for all fq, then G3): tolerant of late w3q
            # chunk DMAs. The 8 live PSUM tiles are spread across all 8
            # banks (4 p1f on ps12, p3f on 2 ps3 + 2 ps12) so no chain
            # start ever waits on a silu/mult eviction.
            for c in range(NCH):
                s1f_tiles = []
                for fq in range(FCH):
                    p1f = ps12.tile([P, NF], F32, tag="ps12",
                                    name=f"p1f_{c}_{fq}")
                    for hp in range(HP):
                        nc.tensor.matmul(
                            p1f[:], w1qc[c][:, hp, fq, :, :], xq[:, hp, :, :],
                            start=(hp == 0), stop=(hp == HP - 1),
                            perf_mode=DRMODE)
                    s1f = s1_pool.tile([P, NF], BF16, tag="s1",
                                       name=f"s1f_{c}_{fq}")
                    nc.scalar.activation(s1f[:], p1f[:], AF.Silu)
                    s1f_tiles.append(s1f)
                for fq in range(FCH):
                    p3f = ps12.tile([P, NF], F32, tag="ps12",
                                    name=f"p3f_{c}_{fq}")
                    for hp in range(HP):
                        nc.tensor.matmul(
                            p3f[:], w3qc[c][:, hp, fq, :, :], xq[:, hp, :, :],
                            start=(hp == 0), stop=(hp == HP - 1),
                            perf_mode=DRMODE)
                    fi = c * FCH + fq
                    nc.vector.tensor_tensor(
                        gf_tiles[fi // 2][:, fi % 2, :], s1f_tiles[fq][:],
                        p3f[:], OP.mult)

            # ---- fp8 phase B: DR chain over 14 f-pairs per h-tile.
            for hh in range(HT):
                pof = ps3.tile([P, NF], F32, tag="ps3", name=f"pof_{hh}")
                for fp_i in range(FP2):
                    nc.tensor.matmul(
                        pof[:], w2qt[:, fp_i, hh, :, :], gf_tiles[fp_i][:],
                        start=(fp_i == 0), stop=(fp_i == FP2 - 1),
                        perf_mode=DRMODE)
                obf = ob_pool.tile([P, NF], F32, tag="ob", name=f"obf_{hh}")
                nc.scalar.copy(obf[:], pof[:])
                nc.sync.dma_start(out=outf_d[hh * P:(hh + 1) * P, :],
                                  in_=obf[:])

    nc.compile()
    return nc


_NC_CACHE = {}


def _get_nc(key, **kw):
    if key not in _NC_CACHE:
        _NC_CACHE[key] = build_moe_nc(**kw)
    return _NC_CACHE[key]


def _host_route(x2, gate_w):
    """Host replay of the router: token lists + routing weights per expert."""
    logits = x2.astype(np.float32) @ gate_w.astype(np.float32).T
    order = np.argsort(-logits, axis=1, kind="stable")[:, :2]
    m = logits.max(axis=1, keepdims=True)
    ex = np.exp(logits - m)
    p = ex / ex.sum(axis=1, keepdims=True)
    T = logits.shape[0]
    p12 = p[np.arange(T)[:, None], order]           # [T, 2]
    p12 = p12 / p12.sum(axis=1, keepdims=True)
    E = gate_w.shape[0]
    idx, wts = [], []
    for e in range(E):
        sel = order == e                             # [T, 2]
        tok = np.nonzero(sel.any(axis=1))[0]
        w = np.where(sel[tok, 0], p12[tok, 0], p12[tok, 1]).astype(np.float32)
        idx.append(tok)
        wts.append(w)
    return idx, wts


def _host_top2_idx(x2, gate_w):
    """Back-compat helper for test.py: token index list per expert."""
    return _host_route(x2, gate_w)[0]


def _select_fp8(idx, wts):
    """Scan bf16 capacity NB: lowest NB whose predicted err <= ERR_TARGET.

    Per expert the lowest-weight (L_e - NB)+ assignments go fp8.
    Returns (NB, NF) padded to 8; NF == 0 means pure bf16.
    """
    loads = np.array([len(i) for i in idx])
    w2tot = sum(float((w.astype(np.float64) ** 2).sum()) for w in wts)
    pref = [np.concatenate([[0.0], np.cumsum(np.sort(w.astype(np.float64) ** 2))])
            for w in wts]
    budget = max(0.0, ERR_TARGET ** 2 - ERR_BASE ** 2)
    lmax = int(loads.max())
    best = (-(-lmax // 8) * 8, 0)
    for nb in range(-(-lmax // 8) * 8 - 8, 0, -8):
        nf = np.maximum(0, loads - nb)
        selw2 = sum(pref[e][min(int(nf[e]), len(wts[e]))]
                    for e in range(len(loads)))
        if w2tot <= 0 or (selw2 / w2tot) * K_ALL3 > budget:
            break
        nfp = -(-int(nf.max()) // 8) * 8
        if nfp > 512:
            break
        best = (nb, nfp)
    return best


def _prep_weights(w1, w2, w3, want_fp8):
    """Per-expert device weight layouts (bf16 + optional fp8 DR packs)."""
    E = w1.shape[0]
    maps = []
    for e in range(E):
        w1t = np.asarray(w1[e], np.float32).T        # [H, F]
        w3t = np.asarray(w3[e], np.float32).T        # [H, F]
        w2t = np.asarray(w2[e], np.float32).T        # [F, H]
        m = {
            "w1c": np.ascontiguousarray(
                w1t.astype(ml_dtypes.bfloat16)
                .reshape(HT, P, NCH, FCH * P).transpose(1, 2, 0, 3)),
            "w3c": np.ascontiguousarray(
                w3t.astype(ml_dtypes.bfloat16)
                .reshape(HT, P, NCH, FCH * P).transpose(1, 2, 0, 3)),
            "w2c": np.ascontiguousarray(
                w2t.astype(ml_dtypes.bfloat16)
                .reshape(FT, P, H).transpose(1, 0, 2)),
        }
        if want_fp8:
            # stationary DR packs: plane i of pair contracts h/f-tile 2k+i
            m["w1q"] = np.ascontiguousarray(
                w1t.astype(E4).reshape(HP, 2, P, NCH, FCH, P)
                .transpose(2, 3, 0, 4, 1, 5))
            m["w3q"] = np.ascontiguousarray(
                w3t.astype(E4).reshape(HP, 2, P, NCH, FCH, P)
                .transpose(2, 3, 0, 4, 1, 5))
            m["w2q"] = np.ascontiguousarray(
                w2t.astype(E4).reshape(FP2, 2, P, HT, P)
                .transpose(2, 0, 3, 1, 4))
        maps.append(m)
    return maps


def kernel(hidden_states, gate_w, w1, w2, w3, _trace=False, _trace_kwargs=None):
    B, S, Hh = hidden_states.shape
    assert Hh == H
    E = gate_w.shape[0]
    T = B * S
    x2 = np.asarray(hidden_states, dtype=np.float32).reshape(T, H)
    idx, wts = _host_route(x2, gate_w)
    xbf = x2.astype(ml_dtypes.bfloat16)

    NB, NF = _select_fp8(idx, wts)
    cmax = max(len(i) for i in idx)
    out = np.zeros((T, H), dtype=np.float32)

    if NF == 0 or NB > CMAXBUILD:
        # fallback: pure-bf16 path (pathological imbalance -> multi-launch)
        wmaps = _prep_weights(w1, w2, w3, want_fp8=False)
        nlaunch = -(-cmax // CMAXBUILD)
        per = -(-cmax // nlaunch)
        C = max(512, -(-per // 8) * 8)
        nc = _get_nc(("bf16", C), C=C)
        for li in range(nlaunch):
            in_maps = []
            for e in range(E):
                tok = idx[e][li * C:(li + 1) * C]
                xg = np.zeros((C, H), dtype=ml_dtypes.bfloat16)
                xg[:len(tok)] = xbf[tok]
                m = dict(wmaps[e])
                m["xt"] = np.ascontiguousarray(
                    xg.T.reshape(HT, P, C).transpose(1, 0, 2))
                in_maps.append(m)
            res = run_bass_kernel_spmd(
                nc, in_maps, list(range(E)), trace=_trace,
                **(_trace_kwargs or {}))
            kernel.last_results = res
            for e, r in enumerate(res.results):
                tok = idx[e][li * C:(li + 1) * C]
                w = wts[e][li * C:(li + 1) * C]
                out[tok] += r["out"][:, :len(tok)].T * w[:, None]
        return out.reshape(B, S, H).astype(hidden_states.dtype)

    wmaps = _prep_weights(w1, w2, w3, want_fp8=True)
    nc = _get_nc(("fp8", NB, NF), C=NB, NF=NF)
    in_maps = []
    hi_sel, lo_sel = [], []
    for e in range(E):
        order_w = np.argsort(wts[e], kind="stable")   # ascending weight
        nf_e = max(0, len(idx[e]) - NB)
        lo = order_w[:nf_e]
        hi = order_w[nf_e:]
        hi_sel.append(hi)
        lo_sel.append(lo)
        xg = np.zeros((NB, H), dtype=ml_dtypes.bfloat16)
        xg[:len(hi)] = xbf[idx[e][hi]]
        xql = np.zeros((NF, H), dtype=E4)
        xql[:len(lo)] = x2[idx[e][lo]].astype(E4)
        m = dict(wmaps[e])
        m["xt"] = np.ascontiguousarray(
            xg.T.reshape(HT, P, NB).transpose(1, 0, 2))
        # xq[p, hp, i, t] = x_lo.T[(2hp+i)*128+p, t]
        m["xq"] = np.ascontiguousarray(
            xql.T.reshape(HP, 2, P, NF).transpose(2, 0, 1, 3))
        in_maps.append(m)
    res = run_bass_kernel_spmd(
        nc, in_maps, list(range(E)), trace=_trace, **(_trace_kwargs or {}))
    kernel.last_results = res
    for e, r in enumerate(res.results):
        hi, lo = hi_sel[e], lo_sel[e]
        if len(hi):
            out[idx[e][hi]] += r["out"][:, :len(hi)].T * wts[e][hi][:, None]
        if len(lo):
            out[idx[e][lo]] += r["outf"][:, :len(lo)].T * wts[e][lo][:, None]
    return out.reshape(B, S, H).astype(hidden_states.dtype)
